# revision 1
# baseline (speedup 1.0000x reference)
# Trainium2 Bass kernel for nn_FMoELinearProj (moe_routing).
#
# Math: all fwd_expert_count values equal max_tokens (=4096), so the ragged
# scatter in the reference is a pure reshape and the whole op is, per expert k:
#     Out[:, k, :] = (X_k @ W_k^T + b_k) @ C_k
#                  = X_k @ (W_k^T C_k) + (b_k @ C_k)
# i.e. ONE [4096,256]x[256,64] GEMM per expert after a tiny on-chip weight
# precompute (W2_k = W_k^T C_k  [256,64],  bc_k = b_k C_k  [64]).
#
# Sharding: expert-parallel, 8 experts per NeuronCore, zero communication.
# Each core reads its token block x[(8m)*4096 : (8m+8)*4096], computes
# out[:, 8m:8m+8, :], host concatenates along axis 1.

import numpy as np

K, TOK, D, E, S, P = 64, 4096, 256, 256, 64, 128
NCORE = 8
KL = K // NCORE          # experts per core
NG = 8                   # token-chunk groups per expert-row sweep
CPG = 4                  # chunks (of 128 tokens) per group; NG*CPG*P = TOK

_CACHE = {}


def _build_nc():
    import concourse.tile as tile
    from concourse import bacc, mybir
    from concourse.masks import make_identity
    from contextlib import ExitStack

    f32 = mybir.dt.float32
    f32r = mybir.dt.float32r

    nc = bacc.Bacc("TRN2", target_bir_lowering=False, debug=False,
                   num_devices=NCORE)
    x_d = nc.dram_tensor("x", [KL * TOK, D], f32, kind="ExternalInput").ap()
    w_d = nc.dram_tensor("w", [KL, E, D], f32, kind="ExternalInput").ap()
    b_d = nc.dram_tensor("b", [KL, E], f32, kind="ExternalInput").ap()
    c_d = nc.dram_tensor("c", [KL, E, S], f32, kind="ExternalInput").ap()
    o_d = nc.dram_tensor("o", [TOK, KL, S], f32, kind="ExternalOutput").ap()

    with tile.TileContext(nc) as tc, ExitStack() as ctx:
        pc = ctx.enter_context(tc.tile_pool(name="consts", bufs=1))
        pw = ctx.enter_context(tc.tile_pool(name="wts", bufs=1))
        px = ctx.enter_context(tc.tile_pool(name="xin", bufs=18))
        pxts = ctx.enter_context(tc.tile_pool(name="xts", bufs=26))
        pst = ctx.enter_context(tc.tile_pool(name="stg", bufs=4))
        ppxt_d = ctx.enter_context(tc.tile_pool(name="ps_xtd", bufs=3, space="PSUM"))
        ppxt_a = ctx.enter_context(tc.tile_pool(name="ps_xta", bufs=3, space="PSUM"))
        ppo = ctx.enter_context(tc.tile_pool(name="ps_o", bufs=2, space="PSUM"))
        ppre = ppo  # preamble psum tiles share the ps_o pool via their own tag

        ident = pc.tile([P, P], f32)
        make_identity(nc, ident)
        ones = pc.tile([1, P], f32)
        nc.gpsimd.memset(ones, 1.0)

        # ---- weight / bias preload -------------------------------------
        w_sb = pw.tile([P, 2, KL, D], f32)    # (p, e-chunk, expert, d)
        c_sb = pw.tile([P, 2, KL, S], f32)    # (p, e-chunk, expert, s)
        b_nat = pw.tile([KL, E], f32)
        w_r = w_d.rearrange("j (ec p) d -> ec p j d", p=P)
        c_r = c_d.rearrange("j (ec p) s -> ec p j s", p=P)
        for ec in range(2):
            nc.sync.dma_start(out=w_sb[:, ec], in_=w_r[ec])
            nc.sync.dma_start(out=c_sb[:, ec], in_=c_r[ec])
        nc.sync.dma_start(out=b_nat, in_=b_d)

        # bias transposed onto partitions: b_t[p, ec, j] = b[j, ec*128+p]
        ps_bt = ppre.tile([P, 512], f32, tag="po")
        for ec in range(2):
            nc.tensor.transpose(ps_bt[:, ec * KL:(ec + 1) * KL],
                                b_nat[0:KL, ec * P:(ec + 1) * P],
                                ident[0:KL, 0:KL])
        b_t = pw.tile([P, 2, KL], f32)
        nc.vector.tensor_copy(b_t, ps_bt[:, 0:2 * KL])

        # ---- W2 = W^T C  per expert: [d, s], stored (p, d-chunk, j, s) --
        w2 = pw.tile([P, 2, KL, S], f32r)
        for j in range(KL):
            for dc in range(2):
                ps = ppre.tile([P, 512], f32, tag="po")
                for ec in range(2):
                    nc.tensor.matmul(ps[:, 0:S],
                                     lhsT=w_sb[:, ec, j, dc * P:(dc + 1) * P],
                                     rhs=c_sb[:, ec, j, :],
                                     start=(ec == 0), stop=(ec == 1))
                nc.vector.tensor_copy(w2[:, dc, j, :], ps[:, 0:S])

        # ---- bc = b C per expert, then broadcast to all 128 partitions --
        bc = pw.tile([1, KL, S], f32)
        for j in range(KL):
            psb = ppre.tile([1, S], f32, tag="po")
            for ec in range(2):
                nc.tensor.matmul(psb,
                                 lhsT=b_t[:, ec, j:j + 1],
                                 rhs=c_sb[:, ec, j, :],
                                 start=(ec == 0), stop=(ec == 1))
            nc.vector.tensor_copy(bc[0:1, j, :], psb)
        psbb = ppre.tile([P, 512], f32, tag="po")
        nc.tensor.matmul(psbb, lhsT=ones[0:1, :], rhs=bc[0:1, :, :],
                         start=True, stop=True)
        bias_bc = pw.tile([P, KL, S], f32)
        nc.vector.tensor_copy(bias_bc, psbb)

        # ---- main loop --------------------------------------------------
        # token t (within expert) = 32*p + n,  n = g*CPG + nl
        x_r = x_d.rearrange("(j p n) d -> j p (n d)", j=KL, p=P)   # [KL,128,8192]
        o_r = o_d.rearrange("(p m) j s -> p (m j s)", p=P)         # [128,16384]
        GSZ_X = CPG * D            # 1024 f32 per partition per group
        GSZ_O = CPG * KL * S       # 2048 f32 per partition per group

        for g in range(NG):
            xg = []
            for j in range(KL):
                t = px.tile([P, CPG, D], f32, tag="xg")
                nc.sync.dma_start(out=t, in_=x_r[j][:, g * GSZ_X:(g + 1) * GSZ_X])
                xg.append(t)
            st = pst.tile([P, CPG, KL, S], f32)
            for nl in range(CPG):
                xts = []
                on_dve = ((g * CPG + nl) % 2 == 0)
                for j in range(KL):
                    pxt = (ppxt_d if on_dve else ppxt_a).tile(
                        [P, 2 * P], f32, tag="xt")
                    nc.tensor.transpose(pxt[:, 0:P], xg[j][:, nl, 0:P], ident)
                    nc.tensor.transpose(pxt[:, P:2 * P], xg[j][:, nl, P:2 * P], ident)
                    xt = pxts.tile([P, 2 * P], f32r, tag="xts")
                    if on_dve:
                        nc.vector.tensor_copy(xt, pxt)
                    else:
                        nc.scalar.copy(xt, pxt)
                    xts.append(xt)
                po = ppo.tile([P, KL, S], f32)
                for j in range(KL):
                    nc.tensor.matmul(po[:, j, :], lhsT=xts[j][:, 0:P],
                                     rhs=w2[:, 0, j, :],
                                     start=(j == 0), stop=False)
                    nc.tensor.matmul(po[:, j, :], lhsT=xts[j][:, P:2 * P],
                                     rhs=w2[:, 1, j, :],
                                     start=False, stop=(j == KL - 1))
                nc.vector.tensor_add(st[:, nl, :, :], po, bias_bc)
            nc.gpsimd.dma_start(out=o_r[:, g * GSZ_O:(g + 1) * GSZ_O], in_=st)
    nc.compile()
    return nc


def _get_nc():
    if "nc" not in _CACHE:
        _CACHE["nc"] = _build_nc()
    return _CACHE["nc"]


def _numpy_fallback(x, counts, w, b, c, mt):
    k = counts.shape[0]
    offs = np.concatenate([[0], np.cumsum(counts)]).astype(np.int64)
    pad = np.zeros((k, mt, x.shape[1]), np.float32)
    for j in range(k):
        cnt = int(counts[j])
        pad[j, :cnt] = x[offs[j]:offs[j] + cnt]
    y = np.einsum("ktd,ked->kte", pad, w) + b[:, None, :]
    valid = (np.arange(mt)[None, :] < counts[:, None])[..., None]
    y = np.where(valid, y, 0.0).transpose(1, 0, 2)
    return np.einsum("nkd,kds->nks", y, c).astype(np.float32)


def kernel(inp, fwd_expert_count, weight, bias, c_psuedo_inv, max_tokens):
    x = np.ascontiguousarray(np.asarray(inp, dtype=np.float32))
    w = np.ascontiguousarray(np.asarray(weight, dtype=np.float32))
    b = np.ascontiguousarray(np.asarray(bias, dtype=np.float32))
    c = np.ascontiguousarray(np.asarray(c_psuedo_inv, dtype=np.float32))
    counts = np.asarray(fwd_expert_count)
    mt = int(max_tokens)

    shapes_ok = (w.shape == (K, E, D) and c.shape == (K, E, S)
                 and b.shape == (K, E) and x.shape == (K * TOK, D)
                 and mt == TOK and bool((counts == mt).all()))
    if not shapes_ok:
        return _numpy_fallback(x, counts, w, b, c, mt)

    from concourse.bass_utils import run_bass_kernel_spmd
    nc = _get_nc()
    in_maps = []
    for m in range(NCORE):
        js = slice(m * KL, (m + 1) * KL)
        in_maps.append({
            "x": x[m * KL * TOK:(m + 1) * KL * TOK],
            "w": w[js],
            "b": b[js],
            "c": c[js],
        })
    res = run_bass_kernel_spmd(nc, in_maps, core_ids=list(range(NCORE)))
    out = np.concatenate([r["o"] for r in res.results], axis=1)
    return np.ascontiguousarray(out.astype(np.float32))



# revision 2
# speedup vs baseline: 2.6037x; 2.6037x over previous
# Trainium2 Bass kernel for nn_FMoELinearProj (moe_routing).
#
# Math: all fwd_expert_count values equal max_tokens (=4096), so the ragged
# scatter in the reference is a pure reshape and the whole op is, per expert k:
#     Out[:, k, :] = (X_k @ W_k^T + b_k) @ C_k
#                  = X_k @ (W_k^T C_k) + (b_k @ C_k)
# i.e. ONE [4096,256]x[256,64] GEMM per expert. The weight fold
# (W2_k = W_k^T C_k [256,64], bc_k = b_k C_k [64]) is ~0.5% of the FLOPs and
# runs on the host, as does the X transpose into the [d, token] layout the
# tensor engine wants and the bf16 casts (rel tolerance is 2e-2; bf16 keeps
# us ~5e-3).
#
# Device (per core, 8 experts): W2 is the stationary matmul operand
# ([128 d, 64 s] per d-chunk), X^T streams as [128 d, 512 tok] moving tiles.
# Two experts are column-tiled side-by-side in the PE array (tile_position
# (0,0)/(0,64)) so their N=512 matmuls run concurrently and fill one PSUM
# bank [128, 512]. One DVE tensor_scalar_add per token block adds the folded
# bias and downcasts psum f32 -> bf16 staging; output is written [j, s, t]
# bf16 and the host transposes/upcasts to the final [t, k, s] f32.
#
# Sharding: expert-parallel, 8 experts per NeuronCore, zero communication.

import numpy as np

K, TOK, D, E, S, P = 64, 4096, 256, 256, 64, 128
NCORE = 8
KL = K // NCORE          # experts per core
DC = D // P              # d-chunks (contraction split), = 2
TB = 512                 # tokens per matmul (moving-operand N)
NTB = TOK // TB          # token blocks per expert, = 8
NJP = KL // 2            # expert pairs per core, = 4

_CACHE = {}


def _build_nc():
    import concourse.tile as tile
    from concourse import bacc, mybir
    from contextlib import ExitStack

    f32 = mybir.dt.float32
    bf16 = mybir.dt.bfloat16

    nc = bacc.Bacc("TRN2", target_bir_lowering=False, debug=False,
                   num_devices=NCORE)
    xt_d = nc.dram_tensor("xt", [DC, P, KL, TOK], bf16, kind="ExternalInput").ap()
    w2_d = nc.dram_tensor("w2", [P, DC, KL, S], bf16, kind="ExternalInput").ap()
    bc_d = nc.dram_tensor("bc", [P, NJP], f32, kind="ExternalInput").ap()
    o_d = nc.dram_tensor("o", [KL, S, TOK], bf16, kind="ExternalOutput").ap()
    o_r = o_d.rearrange("(jj two) s t -> jj (two s) t", two=2)  # [NJP,128,TOK]

    with tile.TileContext(nc) as tc, ExitStack() as ctx:
        pc = ctx.enter_context(tc.tile_pool(name="consts", bufs=1))
        px = ctx.enter_context(tc.tile_pool(name="xin", bufs=8))
        pst = ctx.enter_context(tc.tile_pool(name="stg", bufs=2))
        pp = ctx.enter_context(tc.tile_pool(name="ps", bufs=6, space="PSUM"))

        w2_sb = pc.tile([P, DC, KL, S], bf16)
        nc.sync.dma_start(out=w2_sb, in_=w2_d)
        bc_sb = pc.tile([P, NJP], f32)
        nc.sync.dma_start(out=bc_sb, in_=bc_d)

        for jp in range(NJP):
            j0, j1 = 2 * jp, 2 * jp + 1
            xt = {}
            for j in (j0, j1):
                for dc in range(DC):
                    t = px.tile([P, TOK], bf16, tag="xt")
                    nc.sync.dma_start(out=t, in_=xt_d[dc][:, j, :])
                    xt[(dc, j)] = t
            st = pst.tile([P, TOK], bf16, tag="st")
            for tb in range(NTB):
                sl = slice(tb * TB, (tb + 1) * TB)
                po = pp.tile([P, TB], f32, tag="po")
                nc.tensor.matmul(po[0:S], lhsT=w2_sb[:, 0, j0, :],
                                 rhs=xt[(0, j0)][:, sl], start=True, stop=False)
                nc.tensor.matmul(po[0:S], lhsT=w2_sb[:, 1, j0, :],
                                 rhs=xt[(1, j0)][:, sl], start=False, stop=True)
                nc.tensor.matmul(po[S:P], lhsT=w2_sb[:, 0, j1, :],
                                 rhs=xt[(0, j1)][:, sl], start=True, stop=False)
                nc.tensor.matmul(po[S:P], lhsT=w2_sb[:, 1, j1, :],
                                 rhs=xt[(1, j1)][:, sl], start=False, stop=True)
                nc.vector.tensor_scalar_add(st[:, sl], po, bc_sb[:, jp:jp + 1])
            nc.gpsimd.dma_start(out=o_r[jp], in_=st)
    nc.compile()
    return nc


def _get_nc():
    if "nc" not in _CACHE:
        _CACHE["nc"] = _build_nc()
    return _CACHE["nc"]


def _prep_in_maps(x, w, b, c):
    """Host-side fold + shard: returns run_bass_kernel_spmd in_maps."""
    import ml_dtypes
    bf16 = ml_dtypes.bfloat16

    # W2[k, d, s] = sum_e w[k, e, d] c[k, e, s];  bc[k, s] = sum_e b[k, e] c[k, e, s]
    w2 = np.matmul(w.transpose(0, 2, 1), c)               # [K, D, S] f32
    bc = np.matmul(b[:, None, :], c)[:, 0, :]             # [K, S] f32

    in_maps = []
    for m in range(NCORE):
        js = slice(m * KL, (m + 1) * KL)
        # xt[dc, dl, j, t] = x[(m*KL+j)*TOK + t, dc*128 + dl]  (bf16)
        xm = x[m * KL * TOK:(m + 1) * KL * TOK].astype(bf16)
        xt = np.ascontiguousarray(
            xm.reshape(KL, TOK, DC, P).transpose(2, 3, 0, 1))
        # w2l[dl, dc, j, s] = W2[m*KL+j, dc*128+dl, s]  (bf16)
        w2l = np.ascontiguousarray(
            w2[js].reshape(KL, DC, P, S).transpose(2, 1, 0, 3).astype(bf16))
        # bc2[p, jp]: partitions 0-63 expert 2jp, 64-127 expert 2jp+1 (f32)
        bc2 = np.ascontiguousarray(
            bc[js].reshape(NJP, 2, S).transpose(1, 2, 0).reshape(P, NJP)
            .astype(np.float32))
        in_maps.append({"xt": xt, "w2": w2l, "bc": bc2})
    return in_maps


def _gather_out(results):
    """[KL, S, TOK] bf16 per core -> [TOK, K, S] f32 full output."""
    full = np.concatenate([r["o"] for r in results], axis=0)   # [K, S, TOK]
    return np.ascontiguousarray(full.transpose(2, 0, 1)).astype(np.float32)


def _numpy_fallback(x, counts, w, b, c, mt):
    k = counts.shape[0]
    offs = np.concatenate([[0], np.cumsum(counts)]).astype(np.int64)
    pad = np.zeros((k, mt, x.shape[1]), np.float32)
    for j in range(k):
        cnt = int(counts[j])
        pad[j, :cnt] = x[offs[j]:offs[j] + cnt]
    y = np.einsum("ktd,ked->kte", pad, w) + b[:, None, :]
    valid = (np.arange(mt)[None, :] < counts[:, None])[..., None]
    y = np.where(valid, y, 0.0).transpose(1, 0, 2)
    return np.einsum("nkd,kds->nks", y, c).astype(np.float32)


def kernel(inp, fwd_expert_count, weight, bias, c_psuedo_inv, max_tokens):
    x = np.ascontiguousarray(np.asarray(inp, dtype=np.float32))
    w = np.ascontiguousarray(np.asarray(weight, dtype=np.float32))
    b = np.ascontiguousarray(np.asarray(bias, dtype=np.float32))
    c = np.ascontiguousarray(np.asarray(c_psuedo_inv, dtype=np.float32))
    counts = np.asarray(fwd_expert_count)
    mt = int(max_tokens)

    shapes_ok = (w.shape == (K, E, D) and c.shape == (K, E, S)
                 and b.shape == (K, E) and x.shape == (K * TOK, D)
                 and mt == TOK and bool((counts == mt).all()))
    if not shapes_ok:
        return _numpy_fallback(x, counts, w, b, c, mt)

    from concourse.bass_utils import run_bass_kernel_spmd
    nc = _get_nc()
    in_maps = _prep_in_maps(x, w, b, c)
    res = run_bass_kernel_spmd(nc, in_maps, core_ids=list(range(NCORE)))
    return _gather_out(res.results)
